# revision 5
# baseline (speedup 1.0000x reference)
"""Trainium2 Bass kernel for nn_Concat_26147760898611.

Mean-pool over the word dim of article_concat [256, 2048, 300] and
options_concat [256, 64, 300], concat features -> [256, 600].

Sharding: pure data parallel over batch across 8 NeuronCores
(32 batches per core).  The kernel is HBM-read-bound: 81.1 MB per core
streams through all 16 SDMA engines at their ~27 GB/s per-engine limit
(~421 GB/s aggregate), so the only recoverable time is at the edges.

Per core:
  - each article batch [2048, 300] is DMA'd as one 2.46 MB transfer into
    an SBUF tile [128 partitions, 16 words, 300 feat]; partition p holds
    16 consecutive words (fully contiguous 19.2 KB per partition).
  - the word axis is folded twice on the VectorEngine.  Fold 1 is
    fp32+fp32->fp32 (exact); fold 2 writes bf16 (one rounding of the
    4-word chunk sums, ~1e-4 relative on the final mean).  The surviving
    4 chunks are reduced across the partition dim on the TensorEngine in
    bf16 - ONE PE pass per chunk instead of fp32's two half-speed
    passes, halving PE load so tail matmuls never queue.
  - the article selector (a ones-column that routes each batch's sum
    into its PSUM row via a sliding window) is built on-chip with
    memsets on GpSimd, so no DMA precedes the first data DMA.
  - batches are split 28/4 across two PSUM banks: bank A (rows 0..27)
    is drained (scaled+stored) while batches 28..31 still stream; the
    tail bank's rows live at partition 0 of a dedicated [4, 600] tile
    (compute engines must access partitions at aligned starts), so the
    last activate+store is tiny and issued from the Scalar engine right
    after the final matmul.  Output stores never touch the Sync queue -
    a store there would stall the sequencer's descriptor generation for
    the remaining article DMAs.
  - the last batch is split into geometrically shrinking chunks so the
    post-last-DMA tail is short; its two single-word chunks skip the
    fold and use fp32 matmuls directly.

Self-contained: hardcodes all shapes; no file reads.
"""

import numpy as np

N_CORES = 8
B = 256  # full batch
BC = B // N_CORES  # 32 batches per core
DIM = 300
AW = 2048  # article words per batch
OW = 64  # options words per batch
P = 128  # SBUF partitions
AWP = AW // P  # 16 article words per partition
TAIL_CHUNKS = [8, 4, 2, 1, 1]  # geometric split of the final batch
BANK_A = 28  # batches 0..27 -> early-drained PSUM bank; 28..31 -> tail bank
DATA_BUFS = 6
FOLD_BUFS = 3

_CACHE = {}


def _build_nc():
    import concourse.bacc as bacc
    import concourse.mybir as mybir
    import concourse.tile as tile

    f32 = mybir.dt.float32
    bf16 = mybir.dt.bfloat16
    nc = bacc.Bacc("TRN2", target_bir_lowering=False, debug=False)

    art = nc.dram_tensor("article", [BC, AW, DIM], f32, kind="ExternalInput")
    opt = nc.dram_tensor("options", [BC, OW, DIM], f32, kind="ExternalInput")
    sel_o = nc.dram_tensor("sel_o", [P, BC], f32, kind="ExternalInput")
    out = nc.dram_tensor("out", [BC, 2 * DIM], f32, kind="ExternalOutput")

    # [BC, 128, 16, 300]: partition p <- words p*16 .. p*16+15 (contiguous)
    art_r = art.ap().rearrange("b (p w) f -> b p w f", p=P)
    # per-partition word view of the last batch: [128, 16, 300]
    art_last = art.ap()[BC - 1].rearrange("(p w) f -> p w f", p=P)
    # [128, 16, 300]: partition p <- 16 consecutive words of batch p//4
    opt_r = opt.ap().rearrange("b (s q) f -> (b s) q f", s=P // BC)

    with tile.TileContext(nc) as tc:
        with (
            tc.tile_pool(name="const", bufs=1) as cpool,
            tc.tile_pool(name="data", bufs=DATA_BUFS) as dpool,
            tc.tile_pool(name="fold", bufs=FOLD_BUFS) as fpool,
            tc.tile_pool(name="bfold", bufs=FOLD_BUFS) as bpool,
            tc.tile_pool(name="outp", bufs=1) as opool,
            tc.tile_pool(name="psum", bufs=1, space="PSUM") as ppool,
        ):
            # first Sync-engine instructions: the big data DMAs
            opt_t = dpool.tile([P, AWP, DIM], f32, tag="data")
            nc.sync.dma_start(opt_t[:], opt_r)
            art_t0 = dpool.tile([P, AWP, DIM], f32, tag="data")
            nc.sync.dma_start(art_t0[:], art_r[0])
            art_t1 = dpool.tile([P, AWP, DIM], f32, tag="data")
            nc.sync.dma_start(art_t1[:], art_r[1])

            # options selector: tiny DMA + on-chip cast to bf16
            sel_of = cpool.tile([P, BC], f32, tag="sel_of")
            nc.sync.dma_start(sel_of[:], sel_o.ap()[:])
            sel_ob = cpool.tile([P, BC], bf16, tag="sel_ob")
            nc.vector.tensor_copy(sel_ob[:], sel_of[:])

            # article selector built on-chip (GpSimd): ones column at 31
            sel_ab = cpool.tile([P, 2 * BC - 1], bf16, tag="sel_ab")
            nc.gpsimd.memset(sel_ab[:], 0.0)
            nc.gpsimd.memset(sel_ab[:, BC - 1 : BC], 1.0)

            psum_b28 = ppool.tile([BANK_A, DIM], f32, tag="psum_b28")
            psum_b4 = ppool.tile([BC - BANK_A, DIM], f32, tag="psum_b4")
            psum_a = ppool.tile([BANK_A, DIM], f32, tag="psum_a")
            psum_t = ppool.tile([BC - BANK_A, DIM], f32, tag="psum_t")

            out_t = opool.tile([BANK_A, 2 * DIM], f32, tag="out")
            out_tail = opool.tile([BC - BANK_A, 2 * DIM], f32, tag="out_tail")

            def fold_chunks(t, nch):
                """Fold an SBUF tile t [P, nch, DIM] down to nch//4 (or 1)
                bf16 chunks; fold 1 fp32-exact, final fold rounds to bf16."""
                if nch >= 4:
                    n = nch // 2
                    a = fpool.tile([P, n, DIM], f32, tag=f"fold_{nch}")
                    nc.vector.tensor_add(a[:], t[:, 0:n, :], t[:, n : 2 * n, :])
                    m = n // 2
                    bt = bpool.tile([P, m, DIM], bf16, tag=f"bfold_{nch}")
                    nc.vector.tensor_add(bt[:], a[:, 0:m, :], a[:, m : 2 * m, :])
                else:  # nch == 2
                    m = 1
                    bt = bpool.tile([P, 1, DIM], bf16, tag="bfold_2")
                    nc.vector.tensor_add(bt[:], t[:, 0:1, :], t[:, 1:2, :])
                return bt, m

            def reduce_block(t, nch, sel_b, psum, first, last):
                bt, m = fold_chunks(t, nch)
                for j in range(m):
                    nc.tensor.matmul(
                        psum[:],
                        sel_b,
                        bt[:, j, :],
                        start=(first and j == 0),
                        stop=(last and j == m - 1),
                    )

            def load_reduce(src_ap, nch, sel_b, psum, first, last):
                t = dpool.tile([P, nch, DIM], f32, tag="data")
                nc.sync.dma_start(t[:], src_ap)
                reduce_block(t, nch, sel_b, psum, first, last)

            # options: each folded chunk reduces rows 0..27 and 28..31 in
            # two bf16 matmuls (column slices of the block selector)
            obt, om = fold_chunks(opt_t, AWP)
            for j in range(om):
                nc.tensor.matmul(
                    psum_b28[:], sel_ob[:, 0:BANK_A], obt[:, j, :],
                    start=(j == 0), stop=(j == om - 1),
                )
            for j in range(om):
                nc.tensor.matmul(
                    psum_b4[:], sel_ob[:, BANK_A:BC], obt[:, j, :],
                    start=(j == 0), stop=(j == om - 1),
                )
            nc.scalar.mul(out_t[:, DIM : 2 * DIM], psum_b28[:], 1.0 / OW)
            nc.scalar.mul(out_tail[:, DIM : 2 * DIM], psum_b4[:], 1.0 / OW)

            # articles 0..27 -> bank A (28-wide selector window, ones at b)
            reduce_block(
                art_t0, AWP, sel_ab[:, BC - 1 : BC - 1 + BANK_A], None,
                psum_a, True, False,
            )
            reduce_block(
                art_t1, AWP, sel_ab[:, BC - 2 : BC - 2 + BANK_A], None,
                psum_a, False, False,
            )
            for b in range(2, BANK_A):
                load_reduce(
                    art_r[b],
                    AWP,
                    sel_ab[:, BC - 1 - b : BC - 1 - b + BANK_A],
                    None,
                    psum_a,
                    False,
                    b == BANK_A - 1,
                )
            # bank A drain: scale+store rows 0..27 while 28..31 stream.
            # Issued on Scalar so the Sync sequencer never stalls.
            nc.scalar.mul(out_t[:, 0:DIM], psum_a[:], 1.0 / AW)
            nc.scalar.dma_start(out.ap()[0:BANK_A, :], out_t[:])

            # articles 28..30 -> tail bank (4-wide selector window)
            for b in range(BANK_A, BC - 1):
                load_reduce(
                    art_r[b],
                    AWP,
                    sel_ab[:, BC - 1 - b + BANK_A : BC - 1 - b + BC],
                    None,
                    psum_t,
                    b == BANK_A,
                    False,
                )
            # final batch in geometrically shrinking chunks -> the very
            # last DMA is tiny and its fold+matmul tail is short
            sel_tb = sel_ab[:, BANK_A : BC]
            sel_tf = sel_af[:, BANK_A : BC]
            assert sum(TAIL_CHUNKS) == AWP
            w0 = 0
            for i, nch in enumerate(TAIL_CHUNKS):
                load_reduce(
                    art_last[:, w0 : w0 + nch, :],
                    nch,
                    sel_tb,
                    sel_tf,
                    psum_t,
                    False,
                    i == len(TAIL_CHUNKS) - 1,
                )
                w0 += nch

            # tiny tail drain: Scalar engine scales rows 28..31 and issues
            # the 9.6 KB store itself (HWDGE) with no cross-engine hop
            nc.scalar.mul(out_tail[:, 0:DIM], psum_t[:], 1.0 / AW)
            nc.scalar.dma_start(out.ap()[BANK_A:BC, :], out_tail[:])

    nc.compile()
    return nc


def get_nc():
    if "nc" not in _CACHE:
        _CACHE["nc"] = _build_nc()
    return _CACHE["nc"]


def _sel_arrays():
    sel_o = np.zeros((P, BC), np.float32)
    sel_o[np.arange(P), np.arange(P) // (P // BC)] = 1.0
    return sel_o


def make_in_maps(article, options):
    article = np.ascontiguousarray(np.asarray(article, dtype=np.float32))
    options = np.ascontiguousarray(np.asarray(options, dtype=np.float32))
    assert article.shape == (B, AW, DIM), article.shape
    assert options.shape == (B, OW, DIM), options.shape
    sel_o = _sel_arrays()
    return [
        {
            "article": article[i * BC : (i + 1) * BC],
            "options": options[i * BC : (i + 1) * BC],
            "sel_o": sel_o,
        }
        for i in range(N_CORES)
    ]


def run_sharded(article, options, **spmd_kwargs):
    from concourse.bass_utils import run_bass_kernel_spmd

    nc = get_nc()
    in_maps = make_in_maps(article, options)
    res = run_bass_kernel_spmd(nc, in_maps, list(range(N_CORES)), **spmd_kwargs)
    full = np.concatenate(
        [res.results[i]["out"] for i in range(N_CORES)], axis=0
    ).astype(np.float32)
    return full, res


def kernel(article_concat, options_concat):
    full, _ = run_sharded(article_concat, options_concat)
    return full


# revision 6
# speedup vs baseline: 1.1618x; 1.1618x over previous
"""Trainium2 Bass kernel for nn_Concat_26147760898611.

Mean-pool over the word dim of article_concat [256, 2048, 300] and
options_concat [256, 64, 300], concat features -> [256, 600].

Sharding: pure data parallel over batch across 8 NeuronCores
(32 batches per core).  The kernel is HBM-read-bound: 81.1 MB per core
streams through all 16 SDMA engines at their ~27 GB/s per-engine limit
(~421 GB/s aggregate), so the only recoverable time is at the edges.

Per core:
  - each article batch [2048, 300] is DMA'd as one 2.46 MB transfer into
    an SBUF tile [128 partitions, 16 words, 300 feat]; partition p holds
    16 consecutive words (fully contiguous 19.2 KB per partition).
  - the word axis is folded twice on the VectorEngine (fp32-exact adds);
    the surviving 4 chunks are reduced across the partition dim on the
    TensorEngine with a ones-column selector that routes each batch's
    sum into its PSUM row via a sliding window.  The bulk (batches
    0..27 + options) uses fp32 matmuls: the near-saturated PE keeps the
    power governor from downclocking, which measurably keeps the DMA
    stream at line rate.  The last 4 batches and the final-batch tail
    chunks round their fold-2 output to bf16 and use single-pass bf16
    matmuls, so the post-last-DMA chain drains in ~1 pass per chunk.
  - the article selectors are built on-chip with GpSimd memsets, so no
    DMA precedes the first data DMAs on the Sync queue.
  - batches are split 28/4 across two PSUM banks: bank A (rows 0..27)
    is scaled into SBUF while batches 28..31 still stream; the tail
    bank's 4 rows live at partition 0 of a dedicated [4, 600] tile
    (compute engines need partition-aligned access).  Both output
    stores are issued on Sync AFTER the last data DMA, so the Sync
    sequencer never stalls descriptor generation mid-stream.
  - the last batch is split into geometrically shrinking chunks
    [8,4,2,1,1]; the two single-word chunks are summed in one DVE add
    (fp32+fp32 -> bf16) feeding a single final matmul.

Self-contained: hardcodes all shapes; no file reads.
"""

import numpy as np

N_CORES = 8
B = 256  # full batch
BC = B // N_CORES  # 32 batches per core
DIM = 300
AW = 2048  # article words per batch
OW = 64  # options words per batch
P = 128  # SBUF partitions
AWP = AW // P  # 16 article words per partition
TAIL_CHUNKS = [8, 4, 2]  # geometric split of the final batch head
BANK_A = 28  # batches 0..27 -> early-drained PSUM bank; 28..31 -> tail bank
DATA_BUFS = 6
FOLD_BUFS = 3

_CACHE = {}


def _build_nc():
    import concourse.bacc as bacc
    import concourse.mybir as mybir
    import concourse.tile as tile

    f32 = mybir.dt.float32
    bf16 = mybir.dt.bfloat16
    nc = bacc.Bacc("TRN2", target_bir_lowering=False, debug=False)

    art = nc.dram_tensor("article", [BC, AW, DIM], f32, kind="ExternalInput")
    opt = nc.dram_tensor("options", [BC, OW, DIM], f32, kind="ExternalInput")
    sel_o = nc.dram_tensor("sel_o", [P, BC], f32, kind="ExternalInput")
    out = nc.dram_tensor("out", [BC, 2 * DIM], f32, kind="ExternalOutput")

    # [BC, 128, 16, 300]: partition p <- words p*16 .. p*16+15 (contiguous)
    art_r = art.ap().rearrange("b (p w) f -> b p w f", p=P)
    # per-partition word view of the last batch: [128, 16, 300]
    art_last = art.ap()[BC - 1].rearrange("(p w) f -> p w f", p=P)
    # [128, 16, 300]: partition p <- 16 consecutive words of batch p//4
    opt_r = opt.ap().rearrange("b (s q) f -> (b s) q f", s=P // BC)

    with tile.TileContext(nc) as tc:
        with (
            tc.tile_pool(name="const", bufs=1) as cpool,
            tc.tile_pool(name="data", bufs=DATA_BUFS) as dpool,
            tc.tile_pool(name="fold", bufs=FOLD_BUFS) as fpool,
            tc.tile_pool(name="bfold", bufs=FOLD_BUFS) as bpool,
            tc.tile_pool(name="outp", bufs=1) as opool,
            tc.tile_pool(name="psum", bufs=1, space="PSUM") as ppool,
        ):
            # first Sync-engine instructions: the big data DMAs
            opt_t = dpool.tile([P, AWP, DIM], f32, tag="data")
            nc.sync.dma_start(opt_t[:], opt_r)
            art_t0 = dpool.tile([P, AWP, DIM], f32, tag="data")
            nc.sync.dma_start(art_t0[:], art_r[0])
            art_t1 = dpool.tile([P, AWP, DIM], f32, tag="data")
            nc.sync.dma_start(art_t1[:], art_r[1])

            # options selector (block pattern): tiny DMA, fp32
            sel_of = cpool.tile([P, BC], f32, tag="sel_of")
            nc.sync.dma_start(sel_of[:], sel_o.ap()[:])

            # article selectors built on-chip (GpSimd): ones column at 31
            sel_af = cpool.tile([P, 2 * BC - 1], f32, tag="sel_af")
            nc.gpsimd.memset(sel_af[:], 0.0)
            nc.gpsimd.memset(sel_af[:, BC - 1 : BC], 1.0)
            sel_ab = cpool.tile([P, 2 * BC - 1], bf16, tag="sel_ab")
            nc.gpsimd.memset(sel_ab[:], 0.0)
            nc.gpsimd.memset(sel_ab[:, BC - 1 : BC], 1.0)

            psum_b28 = ppool.tile([BANK_A, DIM], f32, tag="psum_b28")
            psum_b4 = ppool.tile([BC - BANK_A, DIM], f32, tag="psum_b4")
            psum_a = ppool.tile([BANK_A, DIM], f32, tag="psum_a")
            psum_t = ppool.tile([BC - BANK_A, DIM], f32, tag="psum_t")

            out_t = opool.tile([BANK_A, 2 * DIM], f32, tag="out")
            out_tail = opool.tile([BC - BANK_A, 2 * DIM], f32, tag="out_tail")

            def fold_chunks(t, nch, dt):
                """Fold t [P, nch, DIM] down to nch//4 (or 1) chunks; the
                first fold is fp32-exact, the final fold writes dtype dt."""
                if nch >= 4:
                    n = nch // 2
                    a = fpool.tile([P, n, DIM], f32, tag=f"fold_{nch}")
                    nc.vector.tensor_add(a[:], t[:, 0:n, :], t[:, n : 2 * n, :])
                    m = n // 2
                    bt = bpool.tile([P, m, DIM], dt, tag=f"bfold_{nch}_{dt}")
                    nc.vector.tensor_add(bt[:], a[:, 0:m, :], a[:, m : 2 * m, :])
                else:  # nch == 2
                    m = 1
                    bt = bpool.tile([P, 1, DIM], dt, tag=f"bfold_2_{dt}")
                    nc.vector.tensor_add(bt[:], t[:, 0:1, :], t[:, 1:2, :])
                return bt, m

            def reduce_block(t, nch, sel, psum, first, last, dt):
                bt, m = fold_chunks(t, nch, dt)
                for j in range(m):
                    nc.tensor.matmul(
                        psum[:],
                        sel,
                        bt[:, j, :],
                        start=(first and j == 0),
                        stop=(last and j == m - 1),
                    )

            def load_reduce(src_ap, nch, sel, psum, first, last, dt):
                t = dpool.tile([P, nch, DIM], f32, tag="data")
                nc.sync.dma_start(t[:], src_ap)
                reduce_block(t, nch, sel, psum, first, last, dt)

            # options (fp32): each folded chunk reduces rows 0..27 and
            # 28..31 via column slices of the block selector
            obt, om = fold_chunks(opt_t, AWP, f32)
            for j in range(om):
                nc.tensor.matmul(
                    psum_b28[:], sel_of[:, 0:BANK_A], obt[:, j, :],
                    start=(j == 0), stop=(j == om - 1),
                )
            for j in range(om):
                nc.tensor.matmul(
                    psum_b4[:], sel_of[:, BANK_A:BC], obt[:, j, :],
                    start=(j == 0), stop=(j == om - 1),
                )
            nc.scalar.mul(out_t[:, DIM : 2 * DIM], psum_b28[:], 1.0 / OW)
            nc.scalar.mul(out_tail[:, DIM : 2 * DIM], psum_b4[:], 1.0 / OW)

            # articles 0..27 (fp32) -> bank A (28-wide selector window)
            reduce_block(
                art_t0, AWP, sel_af[:, BC - 1 : BC - 1 + BANK_A],
                psum_a, True, False, f32,
            )
            reduce_block(
                art_t1, AWP, sel_af[:, BC - 2 : BC - 2 + BANK_A],
                psum_a, False, False, f32,
            )
            for b in range(2, BANK_A):
                load_reduce(
                    art_r[b],
                    AWP,
                    sel_af[:, BC - 1 - b : BC - 1 - b + BANK_A],
                    psum_a,
                    False,
                    b == BANK_A - 1,
                    f32,
                )
            # bank A drain on Scalar while 28..31 stream
            nc.scalar.mul(out_t[:, 0:DIM], psum_a[:], 1.0 / AW)

            # articles 28..30 (bf16) -> tail bank (4-wide selector window)
            for b in range(BANK_A, BC - 1):
                load_reduce(
                    art_r[b],
                    AWP,
                    sel_ab[:, BC - 1 - b + BANK_A : BC - 1 - b + BC],
                    psum_t,
                    b == BANK_A,
                    False,
                    bf16,
                )
            # final batch: shrinking chunks, all bf16 single-pass matmuls
            sel_tb = sel_ab[:, BANK_A : BC]
            w0 = 0
            for nch in TAIL_CHUNKS:
                load_reduce(
                    art_last[:, w0 : w0 + nch, :], nch, sel_tb,
                    psum_t, False, False, bf16,
                )
                w0 += nch
            # last two words arrive as two tiny DMAs; one DVE add joins
            # them into a single bf16 chunk for the final matmul
            t_w0 = dpool.tile([P, 1, DIM], f32, tag="data")
            nc.sync.dma_start(t_w0[:], art_last[:, w0 : w0 + 1, :])
            t_w1 = dpool.tile([P, 1, DIM], f32, tag="data")
            nc.sync.dma_start(t_w1[:], art_last[:, w0 + 1 : w0 + 2, :])
            bt_l = bpool.tile([P, 1, DIM], bf16, tag="bfold_last")
            nc.vector.tensor_add(bt_l[:], t_w0[:, 0, :], t_w1[:, 0, :])
            nc.tensor.matmul(psum_t[:], sel_tb, bt_l[:, 0, :], start=False, stop=True)

            # stores: issued on Sync after every data DMA, so descriptor
            # generation for the stream is never blocked by a sem wait
            nc.sync.dma_start(out.ap()[0:BANK_A, :], out_t[:])
            nc.scalar.mul(out_tail[:, 0:DIM], psum_t[:], 1.0 / AW)
            nc.sync.dma_start(out.ap()[BANK_A:BC, :], out_tail[:])

    nc.compile()
    return nc


def get_nc():
    if "nc" not in _CACHE:
        _CACHE["nc"] = _build_nc()
    return _CACHE["nc"]


def _sel_arrays():
    sel_o = np.zeros((P, BC), np.float32)
    sel_o[np.arange(P), np.arange(P) // (P // BC)] = 1.0
    return sel_o


def make_in_maps(article, options):
    article = np.ascontiguousarray(np.asarray(article, dtype=np.float32))
    options = np.ascontiguousarray(np.asarray(options, dtype=np.float32))
    assert article.shape == (B, AW, DIM), article.shape
    assert options.shape == (B, OW, DIM), options.shape
    sel_o = _sel_arrays()
    return [
        {
            "article": article[i * BC : (i + 1) * BC],
            "options": options[i * BC : (i + 1) * BC],
            "sel_o": sel_o,
        }
        for i in range(N_CORES)
    ]


def run_sharded(article, options, **spmd_kwargs):
    from concourse.bass_utils import run_bass_kernel_spmd

    nc = get_nc()
    in_maps = make_in_maps(article, options)
    res = run_bass_kernel_spmd(nc, in_maps, list(range(N_CORES)), **spmd_kwargs)
    full = np.concatenate(
        [res.results[i]["out"] for i in range(N_CORES)], axis=0
    ).astype(np.float32)
    return full, res


def kernel(article_concat, options_concat):
    full, _ = run_sharded(article_concat, options_concat)
    return full
